# revision 1
# baseline (speedup 1.0000x reference)
"""Trainium2 Bass kernel for nn_AttentionBlock (sparse sliding-window attention).

Sharding: pure data-parallel over tokens. B=4 batches x 2 T-halves of 512
tokens = 8 shards, one per NeuronCore. Sliding-window(128) causal attention
only needs a 128-token K/V halo per shard, so there is no cross-core
communication at all. Each core runs: QKV projection -> RoPE -> windowed
attention (with per-head sink folded into the softmax denominator) -> output
projection for its 512 tokens.

On-chip layouts (per core):
  xT        [dmodel, 640tok]   (halo 128 + 512 own tokens; halo zero-padded
                                for the first half of each sequence)
  Q.T, K.T  [64feat, tok]      per-head projection output; natural operands
                               for the transposed-scores matmul S.T = K^T Q
  V         [tok, feat]        built by PE-transposing the V.T projection;
                               augmented with a ones column per head so the
                               softmax denominator falls out of the PV matmul
  scores    S.T [key, query]   per (head, 256-query-block, 128-key-block)
  softmax   no max-subtraction (logits are bounded); multiplicative 0/1 mask
            applied after exp; normalization of O.T via gpsimd
            partition_broadcast of 1/denom + elementwise multiply
  out proj  Y [tok, dmodel] accumulated in PSUM over 16 feature tiles with
            the bias folded in as a K=1 rank-1 matmul
All matmuls run as float32r (fp32 storage, full-rate PE mode): weight/x
operands are DMA'd as f32r, compute-produced operands get their f32r
rounding from the last DVE op that writes them.
"""

import math
from contextlib import ExitStack

import numpy as np

import concourse.bacc as bacc
import concourse.tile as tile
from concourse import mybir
from concourse.bass_utils import run_bass_kernel_spmd

_DEBUG = False
F32 = mybir.dt.float32
F32R = mybir.dt.float32r
AF = mybir.ActivationFunctionType
ALU = mybir.AluOpType

B, T, D = 4, 1024, 2048
HEAD_DIM = 64
N_HEADS = 32
N_KV = 8
WINDOW = 128
SM_SCALE = 1.0 / math.sqrt(HEAD_DIM)
ROPE_THETA = 150000.0
SCALING = 32.0
NTK_ALPHA = 1.0
NTK_BETA = 32.0
ICL = 1024

TQ = 512          # queries per shard
HALO = 128
TOK = TQ + HALO   # 640 tokens of K/V context per shard
NKT = D // 128    # 16 contraction tiles over dmodel
NQF = 16          # Q feature tiles (2048 features)
NKF = 4           # K feature tiles (512)
NVF = 4           # V feature tiles (512)
NTT = TOK // 128  # 5 token tiles


def _build_nc():
    nc = bacc.Bacc("TRN2", target_bir_lowering=False, debug=False)

    # ---- DRAM I/O ----
    xt = nc.dram_tensor("xt", (NKT, 128, TOK), F32R, kind="ExternalInput")
    wqk = nc.dram_tensor("wqk", (24, 128, D), F32R, kind="ExternalInput")
    wo = nc.dram_tensor("wo", (16, 128, D), F32R, kind="ExternalInput")
    qkvb = nc.dram_tensor("qkvb", (128, 24), F32, kind="ExternalInput")
    qkvb2 = nc.dram_tensor("qkvb2", (64, 48), F32, kind="ExternalInput")
    outb = nc.dram_tensor("outb", (1, D), F32R, kind="ExternalInput")
    esink = nc.dram_tensor("esink", (1, N_HEADS), F32, kind="ExternalInput")
    costab = nc.dram_tensor("costab", (64, TOK), F32, kind="ExternalInput")
    sintab = nc.dram_tensor("sintab", (64, TOK), F32, kind="ExternalInput")
    mask = nc.dram_tensor("mask", (128, 6, 512), mybir.dt.bfloat16, kind="ExternalInput")
    ident16 = nc.dram_tensor("ident16", (128, 128), mybir.dt.bfloat16, kind="ExternalInput")
    ones1 = nc.dram_tensor("ones1", (1, 128), F32R, kind="ExternalInput")
    pairsel = nc.dram_tensor("pairsel", (128, 2, 128), F32R, kind="ExternalInput")
    ident = nc.dram_tensor("ident", (128, 128), F32, kind="ExternalInput")
    vones = nc.dram_tensor("vones", (128, N_KV, 1), mybir.dt.bfloat16, kind="ExternalInput")
    y = nc.dram_tensor("y", (TQ, D), F32, kind="ExternalOutput")
    if _DEBUG:
        dbg_q = nc.dram_tensor("dbg_q", (64, TQ), F32R, kind="ExternalOutput")
        dbg_k = nc.dram_tensor("dbg_k", (64, TOK), F32R, kind="ExternalOutput")
        dbg_hd = nc.dram_tensor("dbg_hd", (64, TOK), F32, kind="ExternalOutput")
        dbg_v = nc.dram_tensor("dbg_v", (128, 65), F32R, kind="ExternalOutput")
        dbg_pt = nc.dram_tensor("dbg_pt", (128, 256), mybir.dt.bfloat16, kind="ExternalOutput")
        dbg_opk = nc.dram_tensor("dbg_opk", (128, 256), F32, kind="ExternalOutput")
        dbg_rpk = nc.dram_tensor("dbg_rpk", (2, 256), F32, kind="ExternalOutput")
        dbg_rt = nc.dram_tensor("dbg_rt", (128, 256), F32, kind="ExternalOutput")
        dbg_opkn = nc.dram_tensor("dbg_opkn", (128, 256), F32R, kind="ExternalOutput")

    with tile.TileContext(nc) as tc, ExitStack() as ctx:
        ep = ctx.enter_context
        const = ep(tc.tile_pool(name="const", bufs=1))
        hdp = ep(tc.tile_pool(name="hdp", bufs=3))       # pre-rope per-head F32
        swp = ep(tc.tile_pool(name="swp", bufs=2))       # rope swap + product
        ktp = ep(tc.tile_pool(name="ktp", bufs=2 * NKF))  # rotated K (f32r)
        vtp = ep(tc.tile_pool(name="vtp", bufs=2))
        vp = ep(tc.tile_pool(name="vp", bufs=NTT))
        qtp = ep(tc.tile_pool(name="qtp", bufs=4))       # rotated Q (f32r)
        wqkp = ep(tc.tile_pool(name="wqkp", bufs=2))
        opackp = ep(tc.tile_pool(name="opackp", bufs=32))
        opkrawp = ep(tc.tile_pool(name="opkraw", bufs=3))
        rtlp = ep(tc.tile_pool(name="rtlp", bufs=2))
        ptp = ep(tc.tile_pool(name="ptp", bufs=4))
        yp = ep(tc.tile_pool(name="yp", bufs=2))
        rpkp = ep(tc.tile_pool(name="rpkp", bufs=2))
        xctx = ExitStack()
        xtp = xctx.enter_context(tc.tile_pool(name="xtp", bufs=NKT))
        if True:
            # ---- constants ----
            qkvb_sb = const.tile([128, 24], F32)
            nc.sync.dma_start(out=qkvb_sb, in_=qkvb[:])
            qkvb2_sb = const.tile([64, 48], F32)
            nc.sync.dma_start(out=qkvb2_sb, in_=qkvb2[:])
            outb_sb = const.tile([1, D], F32R)
            nc.sync.dma_start(out=outb_sb, in_=outb[:])
            esink_sb = const.tile([1, N_HEADS], F32)
            nc.sync.dma_start(out=esink_sb, in_=esink[:])
            cos_sb = const.tile([64, TOK], F32)
            nc.sync.dma_start(out=cos_sb, in_=costab[:])
            sin_sb = const.tile([64, TOK], F32)
            nc.sync.dma_start(out=sin_sb, in_=sintab[:])
            mask_sb = const.tile([128, 6, 512], mybir.dt.bfloat16)
            nc.sync.dma_start(out=mask_sb, in_=mask[:])
            ones_sb = const.tile([1, 128], F32R)
            nc.sync.dma_start(out=ones_sb, in_=ones1[:])
            pair_sb = const.tile([128, 2, 128], F32R)
            nc.sync.dma_start(out=pair_sb, in_=pairsel[:])
            ident_sb = const.tile([128, 128], F32)
            nc.sync.dma_start(out=ident_sb, in_=ident[:])
            ident16_sb = const.tile([128, 128], mybir.dt.bfloat16)
            nc.sync.dma_start(out=ident16_sb, in_=ident16[:])


            # ---- x.T tiles ----
            wsb_first = wqkp.tile([128, NKT, 128], F32R, tag="wqk")
            nc.sync.dma_start(out=wsb_first, in_=wqk[16])
            xts = []
            for kt in range(NKT):
                t = xtp.tile([128, TOK], F32R, tag="xt")
                nc.sync.dma_start(out=t, in_=xt[kt])
                xts.append(t)

            def rope(hd, dst, tok0, ntok):
                """dst (f32r) = rotary(hd); hd is a [64, ntok] f32 per-head
                tile (rows 0-31 first halves, 32-63 second halves)."""
                swap_t = swp.tile([64, TOK], F32, tag="swap")
                nc.gpsimd.dma_start(out=swap_t[0:32, :ntok], in_=hd[32:64, :])
                nc.gpsimd.dma_start(out=swap_t[32:64, :ntok], in_=hd[0:32, :])
                prod_t = swp.tile([64, TOK], F32, tag="prod")
                nc.vector.tensor_mul(
                    out=prod_t[:, :ntok], in0=hd, in1=cos_sb[:, tok0 : tok0 + ntok]
                )
                nc.vector.tensor_mul(
                    out=swap_t[:, :ntok],
                    in0=swap_t[:, :ntok],
                    in1=sin_sb[:, tok0 : tok0 + ntok],
                )
                nc.vector.tensor_add(
                    out=dst, in0=prod_t[:, :ntok], in1=swap_t[:, :ntok]
                )

            def evac_heads(pst, c0, nt, dst0, dst1, f):
                """Evacuate a [128, nt] projection PSUM chunk into two
                per-head [64, *] F32 SBUF tiles, adding the bias."""
                nc.scalar.activation(
                    out=dst0[:, c0 : c0 + nt],
                    in_=pst[0:64, :],
                    func=AF.Identity,
                    bias=qkvb2_sb[:, 2 * f : 2 * f + 1],
                )
                nc.vector.tensor_scalar(
                    out=dst1[:, c0 : c0 + nt],
                    in0=pst[64:128, :],
                    scalar1=qkvb2_sb[:, 2 * f + 1 : 2 * f + 2],
                    scalar2=None,
                    op0=ALU.add,
                )

            pctx = ExitStack()
            ps512 = pctx.enter_context(tc.tile_pool(name="ps512", bufs=4, space="PSUM"))
            ps256 = pctx.enter_context(tc.tile_pool(name="ps256", bufs=2, space="PSUM"))
            ps65 = pctx.enter_context(tc.tile_pool(name="ps65", bufs=2, space="PSUM"))
            if True:
                # ---- K.T projection (feature tiles 16..19) + RoPE ----
                kth = [None] * N_KV
                for fk in range(NKF):
                    f = 16 + fk
                    if fk == 0:
                        wsb = wsb_first
                    else:
                        wsb = wqkp.tile([128, NKT, 128], F32R, tag="wqk")
                        nc.sync.dma_start(out=wsb, in_=wqk[f])
                    k0 = hdp.tile([64, TOK], F32, tag="hd")
                    k1 = hdp.tile([64, TOK], F32, tag="hd")
                    for c0 in (0, 320):
                        pst_full = ps512.tile([128, 512], F32, tag="ps512")
                        pst = pst_full[:, 0:320]
                        for k in range(NKT):
                            nc.tensor.matmul(
                                pst,
                                wsb[:, k, :],
                                xts[k][:, c0 : c0 + 320],
                                start=(k == 0),
                                stop=(k == NKT - 1),
                            )
                        evac_heads(pst, c0, 320, k0, k1, f)
                    kr0 = ktp.tile([64, TOK], F32R, tag="kt")
                    kr1 = ktp.tile([64, TOK], F32R, tag="kt")
                    rope(k0, kr0, 0, TOK)
                    rope(k1, kr1, 0, TOK)
                    kth[2 * fk] = kr0
                    kth[2 * fk + 1] = kr1
                    if _DEBUG and fk == 0:
                        nc.sync.dma_start(out=dbg_k[:], in_=kr0)
                        nc.sync.dma_start(out=dbg_hd[:], in_=k0)

                # ---- V.T projection (feature tiles 20..23) -> transpose to V ----
                vsbs = []
                for tt in range(NTT):
                    vsb = vp.tile([128, N_KV, 65], mybir.dt.bfloat16, tag="v")
                    nc.gpsimd.dma_start(out=vsb[:, :, 64:65], in_=vones[:])
                    vsbs.append(vsb)
                for fv in range(NVF):
                    f = 20 + fv
                    wsb = wqkp.tile([128, NKT, 128], F32R, tag="wqk")
                    nc.sync.dma_start(out=wsb, in_=wqk[f])
                    vt_sb = vtp.tile([128, TOK], F32, tag="vt")
                    for c0 in (0, 320):
                        pst_full = ps512.tile([128, 512], F32, tag="ps512")
                        pst = pst_full[:, 0:320]
                        for k in range(NKT):
                            nc.tensor.matmul(
                                pst,
                                wsb[:, k, :],
                                xts[k][:, c0 : c0 + 320],
                                start=(k == 0),
                                stop=(k == NKT - 1),
                            )
                        nc.scalar.activation(
                            out=vt_sb[:, c0 : c0 + 320],
                            in_=pst,
                            func=AF.Identity,
                            bias=qkvb_sb[:, f : f + 1],
                        )
                    # transpose each [128,128] block: [2 heads x 64, tok] -> [tok, 2x64]
                    for tt in range(NTT):
                        tps_full = ps256.tile([128, 256], F32, tag="ps256")
                        tps = tps_full[:, 0:128]
                        nc.tensor.transpose(
                            tps, vt_sb[:, tt * 128 : (tt + 1) * 128], ident_sb
                        )
                        nc.vector.tensor_copy(
                            out=vsbs[tt][:, 2 * fv : 2 * fv + 2, 0:64],
                            in_=tps.rearrange("p (h d) -> p h d", h=2),
                        )
                if _DEBUG:
                    nc.sync.dma_start(out=dbg_v[:], in_=vsbs[2][:, 0, :])

                # ---- Q projection + attention, per feature tile (= head pair) ----
                opacks = [[None, None] for _ in range(16)]
                for qf in range(NQF):
                    wsb = wqkp.tile([128, NKT, 128], F32R, tag="wqk")
                    nc.sync.dma_start(out=wsb, in_=wqk[qf])
                    q0 = hdp.tile([64, TQ], F32, tag="hd")
                    q1 = hdp.tile([64, TQ], F32, tag="hd")
                    pst = ps512.tile([128, 512], F32, tag="ps512")
                    for k in range(NKT):
                        nc.tensor.matmul(
                            pst,
                            wsb[:, k, :],
                            xts[k][:, HALO:TOK],
                            start=(k == 0),
                            stop=(k == NKT - 1),
                        )
                    evac_heads(pst, 0, TQ, q0, q1, qf)
                    qrb = qtp.tile([64, 2, TQ], F32R, tag="qt")
                    rope(q0, qrb[:, 0, :], HALO, TQ)
                    rope(q1, qrb[:, 1, :], HALO, TQ)
                    if _DEBUG and qf == 0:
                        nc.sync.dma_start(out=dbg_q[:], in_=qrb[:, 0, :])

                    h = qf // 2  # kv head shared by both q heads in this tile
                    rpk = rpkp.tile([128, 256], F32, tag="rpk")
                    nc.vector.memset(rpk, 1.0)
                    opks = []
                    for qb in range(2):
                        opk = opkrawp.tile([128, 256], F32, tag="opkraw")
                        opks.append(opk)
                        opsum = ps65.tile([65, 512], F32, tag="ps65")
                        for kb in range(3):
                            kcol = qb * 256 + kb * 128
                            stp = ps512.tile([128, 512], F32, tag="ps512")
                            nc.tensor.matmul(
                                stp,
                                kth[h][:, kcol : kcol + 128],
                                qrb[:, :, qb * 256 : qb * 256 + 256],
                                start=True,
                                stop=False,
                            )
                            nc.tensor.matmul(
                                stp,
                                ident16_sb,
                                mask_sb[:, qb * 3 + kb, :],
                                start=False,
                                stop=True,
                            )
                            ptb = ptp.tile([128, 512], mybir.dt.bfloat16, tag="ptb")
                            nc.scalar.activation(
                                out=ptb, in_=stp, func=AF.Exp, scale=SM_SCALE
                            )
                            nc.tensor.matmul(
                                opsum,
                                vsbs[qb * 2 + kb][:, h, :],
                                ptb,
                                start=(kb == 0),
                                stop=(kb == 2),
                            )
                            if _DEBUG and qf == 0 and qb == 0 and kb == 1:
                                nc.sync.dma_start(out=dbg_pt[:], in_=ptb[:, 0:256].bitcast(mybir.dt.bfloat16))
                        for m01 in range(2):
                            qh = 2 * qf + m01
                            rrow = 64 * qb + 32 * m01
                            nc.vector.tensor_scalar(
                                out=rpk[rrow : rrow + 1, :],
                                in0=opsum[64:65, m01 * 256 : m01 * 256 + 256],
                                scalar1=esink_sb[0:1, qh : qh + 1],
                                scalar2=None,
                                op0=ALU.add,
                            )
                        # evacuate unnormalized O.T rows
                        nc.scalar.activation(
                            out=opk[0:64, :], in_=opsum[0:64, 0:256], func=AF.Copy
                        )
                        nc.vector.tensor_copy(
                            out=opk[64:128, :], in_=opsum[0:64, 256:512]
                        )
                    # one reciprocal for all 4 denominator rows of this qf
                    nc.vector.reciprocal(out=rpk, in_=rpk)
                    rpk_r = rtlp.tile([128, 256], F32R, tag="rtile")
                    nc.vector.tensor_copy(out=rpk_r, in_=rpk)
                    for qb in range(2):
                        rps = ps256.tile([128, 256], F32, tag="ps256")
                        nc.tensor.matmul(rps, pair_sb[:, qb, :], rpk_r, start=True, stop=True)
                        opk_n = opackp.tile([128, 256], F32R, tag="opack")
                        nc.vector.tensor_mul(out=opk_n, in0=opks[qb], in1=rps)
                        opacks[qf][qb] = opk_n
                        if _DEBUG and qf == 0 and qb == 0:
                            nc.sync.dma_start(out=dbg_opk[:], in_=opks[0])
                            nc.sync.dma_start(out=dbg_rpk[0:1], in_=rpk[0:1, :])
                            nc.sync.dma_start(out=dbg_rpk[1:2], in_=rpk[32:33, :])
                            nc.sync.dma_start(out=dbg_opkn[:], in_=opk_n)

            # ---- output projection ----
            pctx.close()
            xctx.close()
            wop = ctx.enter_context(tc.tile_pool(name="wop", bufs=6))
            psy = ctx.enter_context(tc.tile_pool(name="psy", bufs=8, space="PSUM"))
            if True:
                for chp in range(2):  # two column-half passes over out_w
                    wos = []
                    for ft in range(16):
                        wosb = wop.tile([128, 1024], F32R, tag="wo")
                        nc.sync.dma_start(
                            out=wosb, in_=wo[ft][:, chp * 1024 : (chp + 1) * 1024]
                        )
                        wos.append(wosb)
                    for tqt in range(4):
                        qb, col = tqt // 2, tqt % 2
                        for c2 in range(2):
                            ch = chp * 2 + c2
                            yps = psy.tile([128, 512], F32, tag="psy")
                            for ft in range(16):
                                nc.tensor.matmul(
                                    yps,
                                    opacks[ft][qb][:, col * 128 : col * 128 + 128],
                                    wos[ft][:, c2 * 512 : c2 * 512 + 512],
                                    start=(ft == 0),
                                    stop=False,
                                )
                            nc.tensor.matmul(
                                yps,
                                ones_sb,
                                outb_sb[:, ch * 512 : ch * 512 + 512],
                                start=False,
                                stop=True,
                            )
                            ysb = yp.tile([128, 512], F32, tag="y")
                            nc.scalar.activation(out=ysb, in_=yps, func=AF.Copy)
                            nc.sync.dma_start(
                                out=y[tqt * 128 : (tqt + 1) * 128, ch * 512 : ch * 512 + 512],
                                in_=ysb,
                            )

    nc.compile()
    return nc


_NC_CACHE = None


def _get_nc():
    global _NC_CACHE
    if _NC_CACHE is None:
        _NC_CACHE = _build_nc()
    return _NC_CACHE


def _rope_tables(positions):
    """fp32 YaRN/NTK-by-parts tables, matching the reference bit-for-bit."""
    d_half = HEAD_DIM // 2
    freq = ROPE_THETA ** (np.arange(0, HEAD_DIM, 2, dtype=np.float32) / HEAD_DIM)
    concentration = 0.1 * math.log(SCALING) + 1.0
    low = d_half * math.log(ICL / (NTK_BETA * 2 * math.pi)) / math.log(ROPE_THETA)
    high = d_half * math.log(ICL / (NTK_ALPHA * 2 * math.pi)) / math.log(ROPE_THETA)
    interpolation = 1.0 / (SCALING * freq)
    extrapolation = 1.0 / freq
    ramp = np.clip(
        (np.arange(d_half, dtype=np.float32) - low) / (high - low), 0.0, 1.0
    )
    inv_freq = interpolation * ramp + extrapolation * (1.0 - ramp)
    freqs = np.outer(positions.astype(np.float32), inv_freq)  # (n, 32)
    return (
        (np.cos(freqs) * concentration).astype(np.float32),
        (np.sin(freqs) * concentration).astype(np.float32),
    )


def _host_inputs(x, qkv_w, qkv_b, out_w, out_b, sinks):
    x = np.asarray(x, np.float32)
    qkv_w = np.asarray(qkv_w, np.float32)
    qkv_b = np.asarray(qkv_b, np.float32)
    out_w = np.asarray(out_w, np.float32)
    out_b = np.asarray(out_b, np.float32)
    sinks = np.asarray(sinks, np.float32)

    wqk_h = np.ascontiguousarray(
        qkv_w.reshape(24, 128, NKT, 128).transpose(0, 3, 2, 1).reshape(24, 128, D)
    )
    wo_h = np.ascontiguousarray(out_w.T.reshape(16, 128, D))
    qkvb_h = np.ascontiguousarray(qkv_b.reshape(24, 128).T)
    qkvb2_h = np.ascontiguousarray(qkv_b.reshape(48, 64).T)
    outb_h = out_b.reshape(1, D).copy()
    esink_h = np.exp(sinks).reshape(1, N_HEADS).astype(np.float32)
    ones_h = np.ones((1, 128), np.float32)
    ident_h = np.eye(128, dtype=np.float32)
    import ml_dtypes
    vones_h = np.ones((128, N_KV, 1), ml_dtypes.bfloat16)
    ident16_h = np.eye(128, dtype=ml_dtypes.bfloat16)
    pair_h = np.zeros((128, 2, 128), np.float32)
    pair_h[0, 0, 0:64] = 1.0
    pair_h[32, 0, 64:128] = 1.0
    pair_h[64, 1, 0:64] = 1.0
    pair_h[96, 1, 64:128] = 1.0

    # masks / rope tables per T-half
    masks, tabs = [], []
    for half in range(2):
        t0 = half * TQ
        p = np.arange(128)[:, None]
        r = np.arange(256)[None, :]
        m = np.zeros((128, 6, 256), np.float32)
        for qb in range(2):
            for kb in range(3):
                dd = kb * 128 + p - r
                vis = (dd >= 1) & (dd <= 128)
                if half == 0:
                    vis = vis & ((qb * 256 + kb * 128 + p) >= HALO)
                m[:, qb * 3 + kb, :] = vis.astype(np.float32)
        import ml_dtypes
        madd = np.where(np.concatenate([m, m], axis=2) > 0.5, 0.0, -1e30)
        masks.append(madd.astype(ml_dtypes.bfloat16))
        pos = np.clip(np.arange(t0 - HALO, t0 + TQ), 0, None)
        cos_t, sin_t = _rope_tables(pos)  # (TOK, 32)
        cos2 = np.concatenate([cos_t.T, cos_t.T], axis=0)       # (64, TOK)
        sin2 = np.concatenate([-sin_t.T, sin_t.T], axis=0)      # (64, TOK) signed
        tabs.append((np.ascontiguousarray(cos2), np.ascontiguousarray(sin2)))

    in_maps = []
    for core in range(8):
        b, half = core // 2, core % 2
        t0 = half * TQ
        x_pad = np.zeros((TOK, D), np.float32)
        lo = t0 - HALO
        x_pad[max(0, -lo) :] = x[b, max(lo, 0) : t0 + TQ]
        xt_h = np.ascontiguousarray(x_pad.T.reshape(NKT, 128, TOK))
        in_maps.append(
            {
                "xt": xt_h,
                "wqk": wqk_h,
                "wo": wo_h,
                "qkvb": qkvb_h,
                "qkvb2": qkvb2_h,
                "outb": outb_h,
                "esink": esink_h,
                "costab": tabs[half][0],
                "sintab": tabs[half][1],
                "mask": masks[half],
                "ones1": ones_h,
                "ident": ident_h,
                "ident16": ident16_h,
                "vones": vones_h,
                "pairsel": pair_h,
            }
        )
    return in_maps


def kernel(x, qkv_w, qkv_b, out_w, out_b, sinks, _trace=False, _tmpdir=None):
    nc = _get_nc()
    in_maps = _host_inputs(x, qkv_w, qkv_b, out_w, out_b, sinks)
    kwargs = {}
    if _trace:
        kwargs = dict(trace=True, tmpdir=_tmpdir)
    res = run_bass_kernel_spmd(nc, in_maps, core_ids=list(range(8)), **kwargs)
    out = np.empty((B, T, D), np.float32)
    for core in range(8):
        b, half = core // 2, core % 2
        out[b, half * TQ : half * TQ + TQ] = res.results[core]["y"]
    if _trace:
        kernel._last_results = res
    return out



# revision 18
# speedup vs baseline: 2.0599x; 2.0599x over previous
"""Trainium2 Bass kernel for nn_AttentionBlock (sliding-window GQA, gpt-oss style).

Sharding: pure data-parallel over tokens. B=4 batches x 2 T-halves of 512
tokens = 8 shards, one per NeuronCore; the 128-token sliding window means each
shard only needs a 128-token K/V halo, so there is no cross-core traffic.

v2 (this file) vs the v1 baseline:
  * every matmul operand is bf16 (f32 PSUM accumulate) -> half the weight DMA
    and full-rate PE everywhere, incl. the small attention tiles.
  * attention uses 128-query x two-128-key triangle blocks (prev + diag).
    Per (head-pair, 128-query block) the scores psum is [128k, (kb, h, q)] and
    the visible set is exactly the two complementary triangles, applied as a
    multiplicative bf16 0/1 mask on the exp'd tile (DVE), so no PE cycles are
    burned on additive mask matmuls.
  * PV runs per head with tile_position=(0,0)/(0,64) so the two heads' O.T
    land stacked in one [128=(2h x 64vf), 128q] psum tile -- which is exactly
    the lhsT layout the out-projection wants; the normalize multiply is the
    evacuation (one DVE op per unit).
  * softmax denominators: ones-column matmuls into a [1, (h,q)] psum row,
    DMA-gathered 4 rows at a time into a per-head-pair [8, 128] tile, one
    reciprocal_approx_fast per head-pair, then broadcast back to 64 vf
    partitions with a tiny selector matmul ([8,128] x [8,128] -> [128,128]).
  * per-head attention sinks are folded in as a per-partition tensor_scalar
    add on the packed denominator tile.

On-chip layouts (per core):
  xT        [dmodel, 640tok] bf16 (halo 128 + 512 own; halo zero-padded on
                                   the first half of each sequence)
  K.T       [64, 640] bf16 per kv head (rotated)
  Q.T       [64, 2, 512] bf16 per head pair (rotated)
  V         [128tok, 8kv, 64] bf16 per 128-token chunk (direct [tok, feat]
            projection: lhsT = xT tile, rhs = wv.T tile -- no PE transpose)
  scores    S.T [128key, (kb, h, 128q)] psum; exp on scalar engine; 0/1
            triangle mask on DVE
  out proj  Y [tok, dmodel] accumulated over 16 feature chunks + K=1 bias
            matmul, evacuated f32
"""

import math
from contextlib import ExitStack

import numpy as np

import concourse.bacc as bacc
import concourse.tile as tile
from concourse import mybir
from concourse.bass_utils import run_bass_kernel_spmd

_DEBUG = False
F32 = mybir.dt.float32
BF16 = mybir.dt.bfloat16
AF = mybir.ActivationFunctionType
ALU = mybir.AluOpType

B, T, D = 4, 1024, 2048
HEAD_DIM = 64
N_HEADS = 32
N_KV = 8
WINDOW = 128
SM_SCALE = 1.0 / math.sqrt(HEAD_DIM)
ROPE_THETA = 150000.0
SCALING = 32.0
NTK_ALPHA = 1.0
NTK_BETA = 32.0
ICL = 1024

TQ = 512          # queries per shard
HALO = 128
TOK = TQ + HALO   # 640 tokens of K/V context per shard
NKT = D // 128    # 16 contraction tiles over dmodel
NQF = 16          # Q feature tiles (2048 features = 16 head pairs)
NKF = 4           # K feature tiles (512 features = 4 kv-head pairs)
NVC = TOK // 128  # 5 V token chunks
NQB = TQ // 128   # 4 query blocks per shard


def _build_nc():
    nc = bacc.Bacc("TRN2", target_bir_lowering=False, debug=False)

    # ---- DRAM I/O ----
    xt = nc.dram_tensor("xt", (NKT, 128, TOK), BF16, kind="ExternalInput")
    wq = nc.dram_tensor("wq", (NQF, 128, D), BF16, kind="ExternalInput")
    wk = nc.dram_tensor("wk", (NKF, 128, D), BF16, kind="ExternalInput")
    wv = nc.dram_tensor("wv", (4, 128, D), BF16, kind="ExternalInput")
    wo = nc.dram_tensor("wo", (16, 128, D), BF16, kind="ExternalInput")
    qbias = nc.dram_tensor("qbias", (128, NQF), F32, kind="ExternalInput")
    kbias = nc.dram_tensor("kbias", (128, NKF), F32, kind="ExternalInput")
    vbias = nc.dram_tensor("vbias", (1, 512), BF16, kind="ExternalInput")
    outb = nc.dram_tensor("outb", (1, D), BF16, kind="ExternalInput")
    cosq = nc.dram_tensor("cosq", (128, TQ), BF16, kind="ExternalInput")
    sinq = nc.dram_tensor("sinq", (128, TQ), BF16, kind="ExternalInput")
    cosk = nc.dram_tensor("cosk", (128, TOK), BF16, kind="ExternalInput")
    sink_ = nc.dram_tensor("sink_", (128, TOK), BF16, kind="ExternalInput")
    maskt = nc.dram_tensor("maskt", (128, 512), BF16, kind="ExternalInput")
    mask0 = nc.dram_tensor("mask0", (128, 512), BF16, kind="ExternalInput")
    esinkc = nc.dram_tensor("esinkc", (8, NQF), F32, kind="ExternalInput")
    selqb = nc.dram_tensor("selqb", (8, NQB, 128), BF16, kind="ExternalInput")
    ones128 = nc.dram_tensor("ones128", (128, 1), BF16, kind="ExternalInput")
    onescol = nc.dram_tensor("onescol", (1, 128), BF16, kind="ExternalInput")
    y = nc.dram_tensor("y", (TQ, D), F32, kind="ExternalOutput")
    if _DEBUG:
        dbg_k = nc.dram_tensor("dbg_k", (64, TOK), BF16, kind="ExternalOutput")
        dbg_q = nc.dram_tensor("dbg_q", (64, 2, TQ), BF16, kind="ExternalOutput")
        dbg_v = nc.dram_tensor("dbg_v", (128, 64), BF16, kind="ExternalOutput")
        dbg_pt = nc.dram_tensor("dbg_pt", (128, 512), BF16, kind="ExternalOutput")
        dbg_dn = nc.dram_tensor("dbg_dn", (8, 128), F32, kind="ExternalOutput")
        dbg_opk = nc.dram_tensor("dbg_opk", (128, 128), BF16, kind="ExternalOutput")

    with tile.TileContext(nc) as tc, ExitStack() as ctx:
        ep = ctx.enter_context
        const = ep(tc.tile_pool(name="const", bufs=1))
        wqp = ep(tc.tile_pool(name="wqp", bufs=3))
        wkp = ep(tc.tile_pool(name="wkp", bufs=2))
        wvp = ep(tc.tile_pool(name="wvp", bufs=4))      # holds all wv (4 groups)
        kthp = ep(tc.tile_pool(name="kthp", bufs=8))    # rotated K per kv head
        khdp = ep(tc.tile_pool(name="khdp", bufs=2))
        kswp = ep(tc.tile_pool(name="kswp", bufs=2))
        vsbp = ep(tc.tile_pool(name="vsbp", bufs=NVC))  # V chunks, held
        qhdp = ep(tc.tile_pool(name="qhdp", bufs=2))
        qswp = ep(tc.tile_pool(name="qswp", bufs=2))
        qrbp = ep(tc.tile_pool(name="qrbp", bufs=2))
        ptbep = ep(tc.tile_pool(name="ptbep", bufs=2))
        ptbp = ep(tc.tile_pool(name="ptbp", bufs=3))
        prp = ep(tc.tile_pool(name="prp", bufs=2))
        opknp = ep(tc.tile_pool(name="opknp", bufs=NQF * NQB))  # held for out proj
        drpp = ep(tc.tile_pool(name="drpp", bufs=2))
        dnsp = ep(tc.tile_pool(name="dnsp", bufs=2))
        drsp = ep(tc.tile_pool(name="drsp", bufs=2))
        drrp = ep(tc.tile_pool(name="drrp", bufs=2))
        drbp = ep(tc.tile_pool(name="drbp", bufs=2))
        xtp = ep(tc.tile_pool(name="xtp", bufs=NKT))
        if True:
            # ---- constants ----
            qbias_sb = const.tile([128, NQF], F32)
            nc.sync.dma_start(out=qbias_sb, in_=qbias[:])
            kbias_sb = const.tile([128, NKF], F32)
            nc.sync.dma_start(out=kbias_sb, in_=kbias[:])
            vbias_sb = const.tile([1, 512], BF16)
            nc.sync.dma_start(out=vbias_sb, in_=vbias[:])
            outb_sb = const.tile([1, D], BF16)
            nc.sync.dma_start(out=outb_sb, in_=outb[:])
            cosq_sb = const.tile([128, TQ], BF16)
            nc.sync.dma_start(out=cosq_sb, in_=cosq[:])
            sinq_sb = const.tile([128, TQ], BF16)
            nc.sync.dma_start(out=sinq_sb, in_=sinq[:])
            cosk_sb = const.tile([128, TOK], BF16)
            nc.sync.dma_start(out=cosk_sb, in_=cosk[:])
            sink_sb = const.tile([128, TOK], BF16)
            nc.sync.dma_start(out=sink_sb, in_=sink_[:])
            maskt_sb = const.tile([128, 512], BF16)
            nc.sync.dma_start(out=maskt_sb, in_=maskt[:])
            mask0_sb = const.tile([128, 512], BF16)
            nc.sync.dma_start(out=mask0_sb, in_=mask0[:])
            esinkc_sb = const.tile([8, NQF], F32)
            nc.sync.dma_start(out=esinkc_sb, in_=esinkc[:])
            selqb_sb = const.tile([8, NQB, 128], BF16)
            nc.sync.dma_start(out=selqb_sb, in_=selqb[:])
            ones128_sb = const.tile([128, 1], BF16)
            nc.sync.dma_start(out=ones128_sb, in_=ones128[:])
            onescol_sb = const.tile([1, 128], BF16)
            nc.sync.dma_start(out=onescol_sb, in_=onescol[:])

            # ---- x.T tiles ----
            xts = []
            for kt in range(NKT):
                t = xtp.tile([128, TOK], BF16, tag="xt")
                nc.sync.dma_start(out=t, in_=xt[kt])
                xts.append(t)
            # ---- V weights (all of them, 4 groups of 4 k-tiles) ----
            wvs = []
            for g in range(4):
                t = wvp.tile([128, 4, 512], BF16, tag="wv")
                nc.sync.dma_start(out=t, in_=wv[g].rearrange("p (g f) -> p g f", g=4))
                wvs.append(t)

            pctx = ExitStack()
            pspj = pctx.enter_context(tc.tile_pool(name="pspj", bufs=2, space="PSUM"))
            psst = pctx.enter_context(tc.tile_pool(name="psst", bufs=2, space="PSUM"))
            psop = pctx.enter_context(tc.tile_pool(name="psop", bufs=2, space="PSUM"))
            psrp = pctx.enter_context(tc.tile_pool(name="psrp", bufs=1, space="PSUM"))
            psdn = pctx.enter_context(tc.tile_pool(name="psdn", bufs=1, space="PSUM"))
            if True:
                def rope64(dst, src, srow, swp_t, cos_t, sin_t, n):
                    """dst[64, n] = src[srow:srow+64] * cos + swap * sin (bf16).

                    All tensor_tensor inputs share base partition srow (walrus
                    requires equal SBUF input base partitions); only the
                    output shifts back to base 0."""
                    s = slice(srow, srow + 64)
                    pr = prp.tile([128, TOK], BF16, tag="ropeprod")
                    nc.vector.tensor_mul(
                        out=pr[s, :n], in0=src[s, :], in1=cos_t[s, :n]
                    )
                    nc.vector.tensor_mul(
                        out=swp_t[s, :n], in0=swp_t[s, :n], in1=sin_t[s, :n]
                    )
                    nc.vector.tensor_add(
                        out=dst, in0=pr[s, :n], in1=swp_t[s, :n]
                    )

                def halfswap(dst, src, base, n):
                    """dst[base:base+64] = src rows [base+32:base+64, base:base+32]."""
                    nc.gpsimd.dma_start(
                        out=dst[base : base + 32, :n], in_=src[base + 32 : base + 64, :]
                    )
                    nc.gpsimd.dma_start(
                        out=dst[base + 32 : base + 64, :n], in_=src[base : base + 32, :]
                    )

                # ---- K projection + rope: 8 kv heads as 4 pairs ----
                kth = [None] * N_KV
                for fk in range(NKF):
                    wsb = wkp.tile([128, NKT, 128], BF16, tag="wk")
                    nc.sync.dma_start(out=wsb, in_=wk[fk])
                    khd = khdp.tile([128, TOK], BF16, tag="khd")
                    for c0 in (0, 320):
                        pst = pspj.tile([128, 512], F32, tag="pj")
                        for k in range(NKT):
                            nc.tensor.matmul(
                                pst[:, 0:320],
                                wsb[:, k, :],
                                xts[k][:, c0 : c0 + 320],
                                start=(k == 0),
                                stop=(k == NKT - 1),
                            )
                        nc.scalar.activation(
                            out=khd[:, c0 : c0 + 320],
                            in_=pst[:, 0:320],
                            func=AF.Identity,
                            bias=kbias_sb[:, fk : fk + 1],
                        )
                    ksw = kswp.tile([128, TOK], BF16, tag="ksw")
                    halfswap(ksw, khd, 0, TOK)
                    halfswap(ksw, khd, 64, TOK)
                    for h2 in range(2):
                        kt_t = kthp.tile([64, TOK], BF16, tag="kth")
                        rope64(kt_t, khd, 64 * h2, ksw, cosk_sb, sink_sb, TOK)
                        kth[2 * fk + h2] = kt_t
                if _DEBUG:
                    nc.sync.dma_start(out=dbg_k[:], in_=kth[0])

                # ---- V: direct [tok, feat] projection ----
                vsbs = []
                for c in range(NVC):
                    psv = pspj.tile([128, 512], F32, tag="pj")
                    for k in range(NKT):
                        nc.tensor.matmul(
                            psv,
                            xts[k][:, 128 * c : 128 * c + 128],
                            wvs[k // 4][:, k % 4, :],
                            start=(k == 0),
                            stop=False,
                        )
                    nc.tensor.matmul(
                        psv, onescol_sb, vbias_sb, start=False, stop=True
                    )
                    vsb = vsbp.tile([128, N_KV, 64], BF16, tag="v")
                    nc.scalar.activation(
                        out=vsb,
                        in_=psv.rearrange("p (h d) -> p h d", h=N_KV),
                        func=AF.Copy,
                    )
                    vsbs.append(vsb)
                if _DEBUG:
                    nc.sync.dma_start(out=dbg_v[:], in_=vsbs[2][:, 0, :])

                # ---- Q projection + attention per head pair ----
                opkns = [[None] * NQB for _ in range(NQF)]
                for qf in range(NQF):
                    h_kv = qf // 2
                    wsb = wqp.tile([128, NKT, 128], BF16, tag="wq")
                    nc.sync.dma_start(out=wsb, in_=wq[qf])
                    psq = pspj.tile([128, 512], F32, tag="pj")
                    for k in range(NKT):
                        nc.tensor.matmul(
                            psq,
                            wsb[:, k, :],
                            xts[k][:, HALO:TOK],
                            start=(k == 0),
                            stop=(k == NKT - 1),
                        )
                    qhd = qhdp.tile([128, TQ], BF16, tag="qhd")
                    nc.scalar.activation(
                        out=qhd, in_=psq, func=AF.Identity,
                        bias=qbias_sb[:, qf : qf + 1],
                    )
                    qsw = qswp.tile([128, TQ], BF16, tag="qsw")
                    halfswap(qsw, qhd, 0, TQ)
                    halfswap(qsw, qhd, 64, TQ)
                    qrb = qrbp.tile([64, 2, TQ], BF16, tag="qrb")
                    for h2 in range(2):
                        rope64(qrb[:, h2, :], qhd, 64 * h2, qsw, cosq_sb, sinq_sb, TQ)
                    if _DEBUG and qf == 0:
                        nc.sync.dma_start(out=dbg_q[:], in_=qrb)

                    dn = psdn.tile([1, 512], F32, tag="dn")
                    drp = drpp.tile([8, 128], F32, tag="drp")
                    ops4 = psop.tile([128, NQB, 128], F32, tag="ops")
                    for qb in range(NQB):
                        # scores: prev + diag key blocks
                        st = psst.tile([128, 512], F32, tag="st")
                        qs = qrb[:, :, 128 * qb : 128 * qb + 128]
                        nc.tensor.matmul(
                            st[:, 0:256],
                            kth[h_kv][:, 128 * qb : 128 * qb + 128],
                            qs, start=True, stop=True,
                        )
                        nc.tensor.matmul(
                            st[:, 256:512],
                            kth[h_kv][:, 128 * qb + 128 : 128 * qb + 256],
                            qs, start=True, stop=True,
                        )
                        ptbe = ptbep.tile([128, 512], BF16, tag="ptbe")
                        nc.scalar.activation(
                            out=ptbe, in_=st, func=AF.Exp, scale=SM_SCALE
                        )
                        ptb = ptbp.tile([128, 512], BF16, tag="ptb")
                        nc.vector.tensor_mul(
                            out=ptb, in0=ptbe,
                            in1=(mask0_sb if qb == 0 else maskt_sb),
                        )
                        if _DEBUG and qf == 0 and qb == 1:
                            nc.sync.dma_start(out=dbg_pt[:], in_=ptb)
                        # denominator rows: [1, (h, q)] accumulated over kb
                        dslot = dn[0:1, 256 * (qb % 2) : 256 * (qb % 2) + 256]
                        nc.tensor.matmul(
                            dslot, ones128_sb, ptb[:, 0:256], start=True, stop=False
                        )
                        nc.tensor.matmul(
                            dslot, ones128_sb, ptb[:, 256:512], start=False, stop=True
                        )
                        # PV: per head, stacked halves of one [128, 128] slot
                        for h2 in range(2):
                            nc.tensor.matmul(
                                ops4[64 * h2 : 64 * h2 + 64, qb, :],
                                vsbs[qb][:, h_kv, :],
                                ptb[:, 128 * h2 : 128 * h2 + 128],
                                start=True, stop=False,
                                tile_position=(0, 64 * h2),
                            )
                            nc.tensor.matmul(
                                ops4[64 * h2 : 64 * h2 + 64, qb, :],
                                vsbs[qb + 1][:, h_kv, :],
                                ptb[:, 256 + 128 * h2 : 256 + 128 * h2 + 128],
                                start=False, stop=True,
                                tile_position=(0, 64 * h2),
                            )
                        if qb % 2 == 1:
                            # evac both units' denom rows, gather -> [4, 128]
                            dn_sb = dnsp.tile([1, 512], F32, tag="dnsb")
                            nc.scalar.activation(out=dn_sb, in_=dn, func=AF.Copy)
                            nc.gpsimd.dma_start(
                                out=drp[2 * (qb - 1) : 2 * (qb - 1) + 4, :],
                                in_=dn_sb,
                            )
                    # packed sink-add + reciprocal for this head pair
                    drs = drsp.tile([8, 128], F32, tag="drs")
                    nc.vector.tensor_scalar(
                        out=drs, in0=drp,
                        scalar1=esinkc_sb[:, qf : qf + 1], scalar2=None,
                        op0=ALU.add,
                    )
                    drr = drrp.tile([8, 128], F32, tag="drr")
                    nc.vector.reciprocal_approx_fast(out=drr, in_=drs)
                    drb = drbp.tile([8, 128], BF16, tag="drb")
                    nc.vector.tensor_copy(out=drb, in_=drr)
                    if _DEBUG and qf == 0:
                        nc.sync.dma_start(out=dbg_dn[:], in_=drr)
                    rps4 = psrp.tile([128, NQB, 128], F32, tag="rps")
                    for qb in range(NQB):
                        nc.tensor.matmul(
                            rps4[:, qb, :], selqb_sb[:, qb, :], drb,
                            start=True, stop=True,
                        )
                    # DVE may read only one PSUM input: evac rps first
                    rps_sb = dnsp.tile([128, NQB, 128], BF16, tag="rpssb")
                    nc.scalar.activation(out=rps_sb, in_=rps4, func=AF.Copy)
                    for qb in range(NQB):
                        opkn = opknp.tile([128, 128], BF16, tag="opkn")
                        nc.vector.tensor_mul(
                            out=opkn, in0=ops4[:, qb, :], in1=rps_sb[:, qb, :]
                        )
                        opkns[qf][qb] = opkn
                        if _DEBUG and qf == 0 and qb == 1:
                            nc.sync.dma_start(out=dbg_opk[:], in_=opkn)

            # ---- output projection ----
            pctx.close()
            wop = ctx.enter_context(tc.tile_pool(name="wop", bufs=16))
            yp = ctx.enter_context(tc.tile_pool(name="yp", bufs=2))
            psy = ctx.enter_context(tc.tile_pool(name="psy", bufs=2, space="PSUM"))
            if True:
                for chp in range(2):
                    wos = []
                    for ft in range(16):
                        wosb = wop.tile([128, 1024], BF16, tag="wo")
                        nc.sync.dma_start(
                            out=wosb, in_=wo[ft][:, chp * 1024 : (chp + 1) * 1024]
                        )
                        wos.append(wosb)
                    for qb in range(NQB):
                        for c2 in range(2):
                            ch = chp * 2 + c2
                            yps = psy.tile([128, 512], F32, tag="psy")
                            for ft in range(16):
                                nc.tensor.matmul(
                                    yps,
                                    opkns[ft][qb],
                                    wos[ft][:, c2 * 512 : c2 * 512 + 512],
                                    start=(ft == 0),
                                    stop=False,
                                )
                            nc.tensor.matmul(
                                yps,
                                onescol_sb,
                                outb_sb[:, ch * 512 : ch * 512 + 512],
                                start=False, stop=True,
                            )
                            ysb = yp.tile([128, 512], F32, tag="y")
                            nc.scalar.activation(out=ysb, in_=yps, func=AF.Copy)
                            nc.sync.dma_start(
                                out=y[qb * 128 : (qb + 1) * 128, ch * 512 : ch * 512 + 512],
                                in_=ysb,
                            )

    nc.compile()
    return nc


_NC_CACHE = None


def _get_nc():
    global _NC_CACHE
    if _NC_CACHE is None:
        _NC_CACHE = _build_nc()
    return _NC_CACHE


def _rope_tables(positions):
    """fp32 YaRN/NTK-by-parts tables, matching the reference bit-for-bit."""
    d_half = HEAD_DIM // 2
    freq = ROPE_THETA ** (np.arange(0, HEAD_DIM, 2, dtype=np.float32) / HEAD_DIM)
    concentration = 0.1 * math.log(SCALING) + 1.0
    low = d_half * math.log(ICL / (NTK_BETA * 2 * math.pi)) / math.log(ROPE_THETA)
    high = d_half * math.log(ICL / (NTK_ALPHA * 2 * math.pi)) / math.log(ROPE_THETA)
    interpolation = 1.0 / (SCALING * freq)
    extrapolation = 1.0 / freq
    ramp = np.clip(
        (np.arange(d_half, dtype=np.float32) - low) / (high - low), 0.0, 1.0
    )
    inv_freq = interpolation * ramp + extrapolation * (1.0 - ramp)
    freqs = np.outer(positions.astype(np.float32), inv_freq)  # (n, 32)
    return (
        (np.cos(freqs) * concentration).astype(np.float32),
        (np.sin(freqs) * concentration).astype(np.float32),
    )


def _host_inputs(x, qkv_w, qkv_b, out_w, out_b, sinks):
    import ml_dtypes

    bf16 = ml_dtypes.bfloat16
    x = np.asarray(x, np.float32)
    qkv_w = np.asarray(qkv_w, np.float32)
    qkv_b = np.asarray(qkv_b, np.float32)
    out_w = np.asarray(out_w, np.float32)
    out_b = np.asarray(out_b, np.float32)
    sinks = np.asarray(sinks, np.float32)

    wq_h = np.ascontiguousarray(
        qkv_w[:2048].reshape(16, 128, NKT, 128).transpose(0, 3, 2, 1).reshape(16, 128, D)
    ).astype(bf16)
    wk_h = np.ascontiguousarray(
        qkv_w[2048:2560].reshape(4, 128, NKT, 128).transpose(0, 3, 2, 1).reshape(4, 128, D)
    ).astype(bf16)
    # wv groups: wv[g][d_local, k2*512 + vf] = Wv[vf, 128*(4g+k2) + d_local]
    wv_h = np.ascontiguousarray(
        qkv_w[2560:3072].T.reshape(4, 4, 128, 512).transpose(0, 2, 1, 3).reshape(4, 128, D)
    ).astype(bf16)
    wo_h = np.ascontiguousarray(out_w.T).reshape(16, 128, D).astype(bf16)
    qbias_h = np.ascontiguousarray(qkv_b[:2048].reshape(16, 128).T)
    kbias_h = np.ascontiguousarray(qkv_b[2048:2560].reshape(4, 128).T)
    vbias_h = qkv_b[2560:3072].reshape(1, 512).astype(bf16)
    outb_h = out_b.reshape(1, D).astype(bf16)
    es = np.exp(sinks).reshape(NQF, 2)          # [head pair, h]
    esinkc_h = np.ascontiguousarray(np.tile(es.T, (4, 1)))  # [8, 16] rows (2qb+h)
    selqb_h = np.zeros((8, NQB, 128), np.float32)
    for qb in range(NQB):
        selqb_h[2 * qb, qb, 0:64] = 1.0
        selqb_h[2 * qb + 1, qb, 64:128] = 1.0
    selqb_h = selqb_h.astype(bf16)
    ones128_h = np.ones((128, 1), bf16)
    onescol_h = np.ones((1, 128), bf16)

    r = np.arange(128)[:, None]
    c = np.arange(128)[None, :]
    mprev = (r > c).astype(np.float32)
    mdiag = (r <= c).astype(np.float32)
    maskt_h = np.concatenate([mprev, mprev, mdiag, mdiag], axis=1).astype(bf16)
    mask0_h0 = np.concatenate(
        [np.zeros((128, 256), np.float32), mdiag, mdiag], axis=1
    ).astype(bf16)

    in_maps = []
    for core in range(8):
        b, half = core // 2, core % 2
        t0 = half * TQ
        x_pad = np.zeros((TOK, D), np.float32)
        lo = t0 - HALO
        x_pad[max(0, -lo):] = x[b, max(lo, 0) : t0 + TQ]
        xt_h = np.ascontiguousarray(x_pad.T.reshape(NKT, 128, TOK)).astype(bf16)
        cq, sq = _rope_tables(np.arange(t0, t0 + TQ))
        ck, sk = _rope_tables(np.clip(np.arange(t0 - HALO, t0 + TQ), 0, None))
        in_maps.append(
            {
                "xt": xt_h,
                "wq": wq_h,
                "wk": wk_h,
                "wv": wv_h,
                "wo": wo_h,
                "qbias": qbias_h,
                "kbias": kbias_h,
                "vbias": vbias_h,
                "outb": outb_h,
                "cosq": np.tile(np.concatenate([cq.T, cq.T], 0), (2, 1)).astype(bf16),
                "sinq": np.tile(np.concatenate([-sq.T, sq.T], 0), (2, 1)).astype(bf16),
                "cosk": np.tile(np.concatenate([ck.T, ck.T], 0), (2, 1)).astype(bf16),
                "sink_": np.tile(np.concatenate([-sk.T, sk.T], 0), (2, 1)).astype(bf16),
                "maskt": maskt_h,
                "mask0": mask0_h0 if half == 0 else maskt_h,
                "esinkc": esinkc_h,
                "selqb": selqb_h,
                "ones128": ones128_h,
                "onescol": onescol_h,
            }
        )
    return in_maps


def kernel(x, qkv_w, qkv_b, out_w, out_b, sinks, _trace=False, _tmpdir=None):
    nc = _get_nc()
    in_maps = _host_inputs(x, qkv_w, qkv_b, out_w, out_b, sinks)
    kwargs = {}
    if _trace:
        kwargs = dict(trace=True, tmpdir=_tmpdir)
    res = run_bass_kernel_spmd(nc, in_maps, core_ids=list(range(8)), **kwargs)
    out = np.empty((B, T, D), np.float32)
    for core in range(8):
        b, half = core // 2, core % 2
        out[b, half * TQ : half * TQ + TQ] = res.results[core]["y"]
    if _trace:
        kernel._last_results = res
    return out


# revision 36
# speedup vs baseline: 2.2273x; 1.0813x over previous
"""Trainium2 Bass kernel for nn_AttentionBlock (sliding-window GQA, gpt-oss style).

Sharding: pure data-parallel over tokens. B=4 batches x 2 T-halves of 512
tokens = 8 shards, one per NeuronCore; the 128-token sliding window means each
shard only needs a 128-token K/V halo, so there is no cross-core traffic.

v2 (this file) vs the v1 baseline:
  * every matmul operand is bf16 (f32 PSUM accumulate) -> half the weight DMA
    and full-rate PE everywhere, incl. the small attention tiles.
  * attention uses 128-query x two-128-key triangle blocks (prev + diag).
    Per (head-pair, 128-query block) the scores psum is [128k, (kb, h, q)] and
    the visible set is exactly the two complementary triangles, applied as a
    multiplicative bf16 0/1 mask on the exp'd tile (DVE), so no PE cycles are
    burned on additive mask matmuls.
  * PV runs per head with tile_position=(0,0)/(0,64) so the two heads' O.T
    land stacked in one [128=(2h x 64vf), 128q] psum tile -- which is exactly
    the lhsT layout the out-projection wants; the normalize multiply is the
    evacuation (one DVE op per unit).
  * softmax denominators: ones-column matmuls into a [1, (h,q)] psum row,
    DMA-gathered 4 rows at a time into a per-head-pair [8, 128] tile, one
    reciprocal_approx_fast per head-pair, then broadcast back to 64 vf
    partitions with a tiny selector matmul ([8,128] x [8,128] -> [128,128]).
  * per-head attention sinks are folded in as a per-partition tensor_scalar
    add on the packed denominator tile.

On-chip layouts (per core):
  xT        [dmodel, 640tok] bf16 (halo 128 + 512 own; halo zero-padded on
                                   the first half of each sequence)
  K.T       [64, 640] bf16 per kv head (rotated)
  Q.T       [64, 2, 512] bf16 per head pair (rotated)
  V         [128tok, 8kv, 64] bf16 per 128-token chunk (direct [tok, feat]
            projection: lhsT = xT tile, rhs = wv.T tile -- no PE transpose)
  scores    S.T [128key, (kb, h, 128q)] psum; exp on scalar engine; 0/1
            triangle mask on DVE
  out proj  Y [tok, dmodel] accumulated over 16 feature chunks + K=1 bias
            matmul, evacuated f32
"""

import math
from contextlib import ExitStack

import numpy as np

import concourse.bacc as bacc
import concourse.tile as tile
from concourse import mybir
from concourse.bass_utils import run_bass_kernel_spmd

_DEBUG = False
F32 = mybir.dt.float32
BF16 = mybir.dt.bfloat16
AF = mybir.ActivationFunctionType
ALU = mybir.AluOpType

B, T, D = 4, 1024, 2048
HEAD_DIM = 64
N_HEADS = 32
N_KV = 8
WINDOW = 128
SM_SCALE = 1.0 / math.sqrt(HEAD_DIM)
ROPE_THETA = 150000.0
SCALING = 32.0
NTK_ALPHA = 1.0
NTK_BETA = 32.0
ICL = 1024

TQ = 512          # queries per shard
HALO = 128
TOK = TQ + HALO   # 640 tokens of K/V context per shard
NKT = D // 128    # 16 contraction tiles over dmodel
NQF = 16          # Q feature tiles (2048 features = 16 head pairs)
NKF = 4           # K feature tiles (512 features = 4 kv-head pairs)
NVC = TOK // 128  # 5 V token chunks
NQB = TQ // 128   # 4 query blocks per shard


def _build_nc():
    nc = bacc.Bacc("TRN2", target_bir_lowering=False, debug=False)

    # ---- DRAM I/O ----
    # xt is partition-major: one DMA, 128 contiguous ~20KB descriptors
    xt = nc.dram_tensor("xt", (128, NKT, TOK), BF16, kind="ExternalInput")
    wq = nc.dram_tensor("wq", (NQF, 128, D), BF16, kind="ExternalInput")
    wk = nc.dram_tensor("wk", (NKF, 128, D), BF16, kind="ExternalInput")
    wv = nc.dram_tensor("wv", (4, 128, D), BF16, kind="ExternalInput")
    wo = nc.dram_tensor("wo", (16, 128, D), BF16, kind="ExternalInput")
    qbias = nc.dram_tensor("qbias", (128, NQF), F32, kind="ExternalInput")
    kbias = nc.dram_tensor("kbias", (128, NKF), F32, kind="ExternalInput")
    vbias = nc.dram_tensor("vbias", (1, 512), BF16, kind="ExternalInput")
    outb = nc.dram_tensor("outb", (1, D), BF16, kind="ExternalInput")
    # rope tables: cos duplicated per head; sinA/sinB are the half-shifted
    # signed sin tables so the rotate-half multiply needs no partition swap
    cosq = nc.dram_tensor("cosq", (128, TQ), BF16, kind="ExternalInput")
    sinqa = nc.dram_tensor("sinqa", (128, TQ), BF16, kind="ExternalInput")
    sinqb = nc.dram_tensor("sinqb", (128, TQ), BF16, kind="ExternalInput")
    cosk = nc.dram_tensor("cosk", (128, TOK), BF16, kind="ExternalInput")
    sinka = nc.dram_tensor("sinka", (128, TOK), BF16, kind="ExternalInput")
    sinkb = nc.dram_tensor("sinkb", (128, TOK), BF16, kind="ExternalInput")
    maskt = nc.dram_tensor("maskt", (128, 512), BF16, kind="ExternalInput")
    mask0 = nc.dram_tensor("mask0", (128, 512), BF16, kind="ExternalInput")
    esinkc = nc.dram_tensor("esinkc", (8, NQF), F32, kind="ExternalInput")
    selqb = nc.dram_tensor("selqb", (8, NQB, 128), BF16, kind="ExternalInput")
    ones128 = nc.dram_tensor("ones128", (128, 1), BF16, kind="ExternalInput")
    onescol = nc.dram_tensor("onescol", (1, 128), BF16, kind="ExternalInput")
    y = nc.dram_tensor("y", (TQ, D), F32, kind="ExternalOutput")
    if _DEBUG:
        dbg_k = nc.dram_tensor("dbg_k", (64, TOK), BF16, kind="ExternalOutput")
        dbg_q = nc.dram_tensor("dbg_q", (64, 2, TQ), BF16, kind="ExternalOutput")
        dbg_v = nc.dram_tensor("dbg_v", (128, 64), BF16, kind="ExternalOutput")
        dbg_pt = nc.dram_tensor("dbg_pt", (128, 512), BF16, kind="ExternalOutput")
        dbg_dn = nc.dram_tensor("dbg_dn", (8, 128), F32, kind="ExternalOutput")
        dbg_opk = nc.dram_tensor("dbg_opk", (128, 128), BF16, kind="ExternalOutput")

    with tile.TileContext(nc) as tc, ExitStack() as ctx:
        ep = ctx.enter_context
        const = ep(tc.tile_pool(name="const", bufs=1))
        wqp = ep(tc.tile_pool(name="wqp", bufs=4))
        kthp = ep(tc.tile_pool(name="kthp", bufs=8))    # rotated K per kv head
        vsbp = ep(tc.tile_pool(name="vsbp", bufs=NVC))  # V chunks, held
        qhdp = ep(tc.tile_pool(name="qhdp", bufs=3))
        qrbp = ep(tc.tile_pool(name="qrbp", bufs=3))
        ptbep = ep(tc.tile_pool(name="ptbep", bufs=2))
        ptbp = ep(tc.tile_pool(name="ptbp", bufs=4))
        prp = ep(tc.tile_pool(name="prp", bufs=2))
        pwp = ep(tc.tile_pool(name="pwp", bufs=2))
        opknp = ep(tc.tile_pool(name="opknp", bufs=NQF * NQB))  # held for out proj
        drpp = ep(tc.tile_pool(name="drpp", bufs=2))
        dnsp = ep(tc.tile_pool(name="dnsp", bufs=2))
        rpssp = ep(tc.tile_pool(name="rpssp", bufs=2))
        drsp = ep(tc.tile_pool(name="drsp", bufs=2))
        drrp = ep(tc.tile_pool(name="drrp", bufs=2))
        drbp = ep(tc.tile_pool(name="drbp", bufs=2))
        xtp = ep(tc.tile_pool(name="xtp", bufs=1))
        # K/V-phase-only pools: created last (top of the SBUF pool stack) so
        # they can be released before the wo prefetch reuses their space
        kvctx = ExitStack()
        wkp = kvctx.enter_context(tc.tile_pool(name="wkp", bufs=4))
        wvp = kvctx.enter_context(tc.tile_pool(name="wvp", bufs=4))
        khdp = kvctx.enter_context(tc.tile_pool(name="khdp", bufs=2))
        if True:
            # ---- highest-priority input DMAs first (gpsimd queue):
            # xt (one big partition-contiguous transfer), then K weights
            xts_all = xtp.tile([128, NKT, TOK], BF16, tag="xt")
            nc.gpsimd.dma_start(out=xts_all, in_=xt[:])
            xts = [xts_all[:, kt, :] for kt in range(NKT)]
            wks = []
            for fk in range(NKF):
                t = wkp.tile([128, NKT, 128], BF16, tag="wk")
                nc.gpsimd.dma_start(out=t, in_=wk[fk])
                wks.append(t)
            wvs = []
            for g in range(4):
                t = wvp.tile([128, 4, 512], BF16, tag="wv")
                nc.gpsimd.dma_start(out=t, in_=wv[g].rearrange("p (g f) -> p g f", g=4))
                wvs.append(t)
            # first few Q weight tiles on sync (rest just-in-time in the loop)
            wqs = [None] * NQF
            for qf in range(4):
                wqs[qf] = wqp.tile([128, NKT, 128], BF16, tag="wq", name=f"wq{qf}")
                nc.sync.dma_start(out=wqs[qf], in_=wq[qf])

            # ---- constants (scalar queue, K-phase needs first) ----
            kbias_sb = const.tile([128, NKF], F32)
            nc.scalar.dma_start(out=kbias_sb, in_=kbias[:])
            cosk_sb = const.tile([128, TOK], BF16)
            nc.scalar.dma_start(out=cosk_sb, in_=cosk[:])
            sinka_sb = const.tile([128, TOK], BF16)
            nc.scalar.dma_start(out=sinka_sb, in_=sinka[:])
            sinkb_sb = const.tile([128, TOK], BF16)
            nc.scalar.dma_start(out=sinkb_sb, in_=sinkb[:])
            vbias_sb = const.tile([1, 512], BF16)
            nc.scalar.dma_start(out=vbias_sb, in_=vbias[:])
            onescol_sb = const.tile([1, 128], BF16)
            nc.scalar.dma_start(out=onescol_sb, in_=onescol[:])
            qbias_sb = const.tile([128, NQF], F32)
            nc.scalar.dma_start(out=qbias_sb, in_=qbias[:])
            cosq_sb = const.tile([128, TQ], BF16)
            nc.scalar.dma_start(out=cosq_sb, in_=cosq[:])
            sinqa_sb = const.tile([128, TQ], BF16)
            nc.scalar.dma_start(out=sinqa_sb, in_=sinqa[:])
            sinqb_sb = const.tile([128, TQ], BF16)
            nc.scalar.dma_start(out=sinqb_sb, in_=sinqb[:])
            maskt_sb = const.tile([128, 512], BF16)
            nc.scalar.dma_start(out=maskt_sb, in_=maskt[:])
            mask0_sb = const.tile([128, 512], BF16)
            nc.scalar.dma_start(out=mask0_sb, in_=mask0[:])
            esinkc_sb = const.tile([8, NQF], F32)
            nc.scalar.dma_start(out=esinkc_sb, in_=esinkc[:])
            selqb_sb = const.tile([8, NQB, 128], BF16)
            nc.scalar.dma_start(out=selqb_sb, in_=selqb[:])
            ones128_sb = const.tile([128, 1], BF16)
            nc.scalar.dma_start(out=ones128_sb, in_=ones128[:])
            outb_sb = const.tile([1, D], BF16)
            nc.scalar.dma_start(out=outb_sb, in_=outb[:])

            pctx = ExitStack()
            pspj = pctx.enter_context(tc.tile_pool(name="pspj", bufs=2, space="PSUM"))
            psst = pctx.enter_context(tc.tile_pool(name="psst", bufs=2, space="PSUM"))
            psop = pctx.enter_context(tc.tile_pool(name="psop", bufs=2, space="PSUM"))
            psrp = pctx.enter_context(tc.tile_pool(name="psrp", bufs=1, space="PSUM"))
            psdn = pctx.enter_context(tc.tile_pool(name="psdn", bufs=1, space="PSUM"))
            if True:
                def rope_pair(dsts, src, cos_t, sina_t, sinb_t, n):
                    """Rotate both 64-row heads of src [128, n] into dsts[h]
                    [64, n] without any partition swap: the rotate-half
                    product uses half-shifted sin tables so every
                    tensor_tensor's two inputs share a base partition."""
                    pr = prp.tile([128, TOK], BF16, tag="ropepr")
                    nc.vector.tensor_mul(
                        out=pr[:, :n], in0=src[:, :n], in1=cos_t[:, :n]
                    )
                    pw = pwp.tile([128, TOK], BF16, tag="ropepw")
                    for b0 in (0, 64):
                        nc.vector.tensor_mul(
                            out=pw[b0 : b0 + 32, :n],
                            in0=src[b0 + 32 : b0 + 64, :n],
                            in1=sina_t[b0 + 32 : b0 + 64, :n],
                        )
                        nc.vector.tensor_mul(
                            out=pw[b0 + 32 : b0 + 64, :n],
                            in0=src[b0 : b0 + 32, :n],
                            in1=sinb_t[b0 : b0 + 32, :n],
                        )
                    for h2 in range(2):
                        nc.vector.tensor_add(
                            out=dsts[h2],
                            in0=pr[64 * h2 : 64 * h2 + 64, :n],
                            in1=pw[64 * h2 : 64 * h2 + 64, :n],
                        )

                # ---- K projection + rope: 8 kv heads as 4 pairs ----
                kth = [None] * N_KV
                for fk in range(NKF):
                    wsb = wks[fk]
                    khd = khdp.tile([128, TOK], BF16, tag="khd")
                    for c0 in (0, 320):
                        pst = pspj.tile([128, 512], F32, tag="pj")
                        for k in range(NKT):
                            nc.tensor.matmul(
                                pst[:, 0:320],
                                wsb[:, k, :],
                                xts[k][:, c0 : c0 + 320],
                                start=(k == 0),
                                stop=(k == NKT - 1),
                            )
                        nc.scalar.activation(
                            out=khd[:, c0 : c0 + 320],
                            in_=pst[:, 0:320],
                            func=AF.Identity,
                            bias=kbias_sb[:, fk : fk + 1],
                        )
                    kt0 = kthp.tile([64, TOK], BF16, tag="kth")
                    kt1 = kthp.tile([64, TOK], BF16, tag="kth")
                    rope_pair([kt0, kt1], khd, cosk_sb, sinka_sb, sinkb_sb, TOK)
                    kth[2 * fk] = kt0
                    kth[2 * fk + 1] = kt1
                if _DEBUG:
                    nc.sync.dma_start(out=dbg_k[:], in_=kth[0])

                # ---- V: direct [tok, feat] projection ----
                vsbs = []
                for c in range(NVC):
                    psv = pspj.tile([128, 512], F32, tag="pj")
                    for k in range(NKT):
                        nc.tensor.matmul(
                            psv,
                            xts[k][:, 128 * c : 128 * c + 128],
                            wvs[k // 4][:, k % 4, :],
                            start=(k == 0),
                            stop=False,
                        )
                    nc.tensor.matmul(
                        psv, onescol_sb, vbias_sb, start=False, stop=True
                    )
                    vsb = vsbp.tile([128, N_KV, 64], BF16, tag="v")
                    nc.scalar.activation(
                        out=vsb,
                        in_=psv.rearrange("p (h d) -> p h d", h=N_KV),
                        func=AF.Copy,
                    )
                    vsbs.append(vsb)
                if _DEBUG:
                    nc.sync.dma_start(out=dbg_v[:], in_=vsbs[2][:, 0, :])

                # ---- out-proj weight prefetch (first half) ----
                kvctx.close()  # free wk/wv/khd SBUF for the wo tiles
                wop = ctx.enter_context(tc.tile_pool(name="wop", bufs=32))
                wos = []
                for ft in range(16):
                    wosb = wop.tile([128, 1024], BF16, tag="wo")
                    nc.sync.dma_start(out=wosb, in_=wo[ft][:, 0:1024])
                    wos.append(wosb)

                # ---- Q projection + attention per head pair ----
                opkns = [[None] * NQB for _ in range(NQF)]
                for qf in range(NQF):
                    h_kv = qf // 2
                    if qf + 4 < NQF:
                        wqs[qf + 4] = wqp.tile([128, NKT, 128], BF16, tag="wq", name=f"wq{qf+4}")
                        nc.sync.dma_start(out=wqs[qf + 4], in_=wq[qf + 4])
                    if qf == 11:
                        # prefetch second half of out-proj weights
                        for ft in range(16):
                            wosb = wop.tile([128, 1024], BF16, tag="wo")
                            nc.sync.dma_start(
                                out=wosb, in_=wo[ft][:, 1024:2048]
                            )
                            wos.append(wosb)
                    wsb = wqs[qf]
                    psq = pspj.tile([128, 512], F32, tag="pj")
                    for k in range(NKT):
                        nc.tensor.matmul(
                            psq,
                            wsb[:, k, :],
                            xts[k][:, HALO:TOK],
                            start=(k == 0),
                            stop=(k == NKT - 1),
                        )
                    qhd = qhdp.tile([128, TQ], BF16, tag="qhd")
                    nc.scalar.activation(
                        out=qhd, in_=psq, func=AF.Identity,
                        bias=qbias_sb[:, qf : qf + 1],
                    )
                    qrb = qrbp.tile([64, 2, TQ], BF16, tag="qrb")
                    rope_pair(
                        [qrb[:, 0, :], qrb[:, 1, :]], qhd,
                        cosq_sb, sinqa_sb, sinqb_sb, TQ,
                    )
                    if _DEBUG and qf == 0:
                        nc.sync.dma_start(out=dbg_q[:], in_=qrb)

                    dn = psdn.tile([1, 512], F32, tag="dn")
                    drp = drpp.tile([8, 128], F32, tag="drp")
                    ops4 = psop.tile([128, NQB, 128], F32, tag="ops")
                    for qb in range(NQB):
                        # scores: prev + diag key blocks
                        st = psst.tile([128, 512], F32, tag="st")
                        qs = qrb[:, :, 128 * qb : 128 * qb + 128]
                        nc.tensor.matmul(
                            st[:, 0:256],
                            kth[h_kv][:, 128 * qb : 128 * qb + 128],
                            qs, start=True, stop=True,
                        )
                        nc.tensor.matmul(
                            st[:, 256:512],
                            kth[h_kv][:, 128 * qb + 128 : 128 * qb + 256],
                            qs, start=True, stop=True,
                        )
                        ptbe = ptbep.tile([128, 512], BF16, tag="ptbe")
                        nc.scalar.activation(
                            out=ptbe, in_=st, func=AF.Exp, scale=SM_SCALE
                        )
                        ptb = ptbp.tile([128, 512], BF16, tag="ptb")
                        nc.vector.tensor_mul(
                            out=ptb, in0=ptbe,
                            in1=(mask0_sb if qb == 0 else maskt_sb),
                        )
                        if _DEBUG and qf == 0 and qb == 1:
                            nc.sync.dma_start(out=dbg_pt[:], in_=ptb)
                        # denominator rows: [1, (h, q)] accumulated over kb
                        dslot = dn[0:1, 256 * (qb % 2) : 256 * (qb % 2) + 256]
                        nc.tensor.matmul(
                            dslot, ones128_sb, ptb[:, 0:256], start=True, stop=False
                        )
                        nc.tensor.matmul(
                            dslot, ones128_sb, ptb[:, 256:512], start=False, stop=True
                        )
                        # PV: per head, stacked halves of one [128, 128] slot
                        for h2 in range(2):
                            nc.tensor.matmul(
                                ops4[64 * h2 : 64 * h2 + 64, qb, :],
                                vsbs[qb][:, h_kv, :],
                                ptb[:, 128 * h2 : 128 * h2 + 128],
                                start=True, stop=False,
                                tile_position=(0, 64 * h2),
                            )
                            nc.tensor.matmul(
                                ops4[64 * h2 : 64 * h2 + 64, qb, :],
                                vsbs[qb + 1][:, h_kv, :],
                                ptb[:, 256 + 128 * h2 : 256 + 128 * h2 + 128],
                                start=False, stop=True,
                                tile_position=(0, 64 * h2),
                            )
                        if qb % 2 == 1:
                            # evac both units' denom rows, gather -> [4, 128]
                            dn_sb = dnsp.tile([1, 512], F32, tag="dnsb")
                            nc.scalar.activation(out=dn_sb, in_=dn, func=AF.Copy)
                            nc.gpsimd.dma_start(
                                out=drp[2 * (qb - 1) : 2 * (qb - 1) + 4, :],
                                in_=dn_sb,
                            )
                    # packed sink-add + reciprocal for this head pair
                    drs = drsp.tile([8, 128], F32, tag="drs")
                    nc.vector.tensor_scalar(
                        out=drs, in0=drp,
                        scalar1=esinkc_sb[:, qf : qf + 1], scalar2=None,
                        op0=ALU.add,
                    )
                    drr = drrp.tile([8, 128], F32, tag="drr")
                    nc.vector.reciprocal_approx_fast(out=drr, in_=drs)
                    drb = drbp.tile([8, 128], BF16, tag="drb")
                    nc.vector.tensor_copy(out=drb, in_=drr)
                    if _DEBUG and qf == 0:
                        nc.sync.dma_start(out=dbg_dn[:], in_=drr)
                    rps4 = psrp.tile([128, NQB, 128], F32, tag="rps")
                    for qb in range(NQB):
                        nc.tensor.matmul(
                            rps4[:, qb, :], selqb_sb[:, qb, :], drb,
                            start=True, stop=True,
                        )
                    # DVE may read only one PSUM input: evac rps first
                    rps_sb = rpssp.tile([128, NQB, 128], BF16, tag="rpssb")
                    nc.scalar.activation(out=rps_sb, in_=rps4, func=AF.Copy)
                    for qb in range(NQB):
                        opkn = opknp.tile([128, 128], BF16, tag="opkn")
                        nc.vector.tensor_mul(
                            out=opkn, in0=ops4[:, qb, :], in1=rps_sb[:, qb, :]
                        )
                        opkns[qf][qb] = opkn
                        if _DEBUG and qf == 0 and qb == 1:
                            nc.sync.dma_start(out=dbg_opk[:], in_=opkn)

            # ---- output projection ----
            pctx.close()
            yp = ctx.enter_context(tc.tile_pool(name="yp", bufs=2))
            psy = ctx.enter_context(tc.tile_pool(name="psy", bufs=2, space="PSUM"))
            if True:
                for chp in range(2):
                    for qb in range(NQB):
                        for c2 in range(2):
                            ch = chp * 2 + c2
                            yps = psy.tile([128, 512], F32, tag="psy")
                            for ft in range(16):
                                nc.tensor.matmul(
                                    yps,
                                    opkns[ft][qb],
                                    wos[chp * 16 + ft][:, c2 * 512 : c2 * 512 + 512],
                                    start=(ft == 0),
                                    stop=False,
                                )
                            nc.tensor.matmul(
                                yps,
                                onescol_sb,
                                outb_sb[:, ch * 512 : ch * 512 + 512],
                                start=False, stop=True,
                            )
                            ysb = yp.tile([128, 512], F32, tag="y")
                            nc.scalar.activation(out=ysb, in_=yps, func=AF.Copy)
                            nc.sync.dma_start(
                                out=y[qb * 128 : (qb + 1) * 128, ch * 512 : ch * 512 + 512],
                                in_=ysb,
                            )

    nc.compile()
    return nc


_NC_CACHE = None


def _get_nc():
    global _NC_CACHE
    if _NC_CACHE is None:
        _NC_CACHE = _build_nc()
    return _NC_CACHE


def _rope_tables(positions):
    """fp32 YaRN/NTK-by-parts tables, matching the reference bit-for-bit."""
    d_half = HEAD_DIM // 2
    freq = ROPE_THETA ** (np.arange(0, HEAD_DIM, 2, dtype=np.float32) / HEAD_DIM)
    concentration = 0.1 * math.log(SCALING) + 1.0
    low = d_half * math.log(ICL / (NTK_BETA * 2 * math.pi)) / math.log(ROPE_THETA)
    high = d_half * math.log(ICL / (NTK_ALPHA * 2 * math.pi)) / math.log(ROPE_THETA)
    interpolation = 1.0 / (SCALING * freq)
    extrapolation = 1.0 / freq
    ramp = np.clip(
        (np.arange(d_half, dtype=np.float32) - low) / (high - low), 0.0, 1.0
    )
    inv_freq = interpolation * ramp + extrapolation * (1.0 - ramp)
    freqs = np.outer(positions.astype(np.float32), inv_freq)  # (n, 32)
    return (
        (np.cos(freqs) * concentration).astype(np.float32),
        (np.sin(freqs) * concentration).astype(np.float32),
    )


def _host_inputs(x, qkv_w, qkv_b, out_w, out_b, sinks):
    import ml_dtypes

    bf16 = ml_dtypes.bfloat16
    x = np.asarray(x, np.float32)
    qkv_w = np.asarray(qkv_w, np.float32)
    qkv_b = np.asarray(qkv_b, np.float32)
    out_w = np.asarray(out_w, np.float32)
    out_b = np.asarray(out_b, np.float32)
    sinks = np.asarray(sinks, np.float32)

    wq_h = np.ascontiguousarray(
        qkv_w[:2048].reshape(16, 128, NKT, 128).transpose(0, 3, 2, 1).reshape(16, 128, D)
    ).astype(bf16)
    wk_h = np.ascontiguousarray(
        qkv_w[2048:2560].reshape(4, 128, NKT, 128).transpose(0, 3, 2, 1).reshape(4, 128, D)
    ).astype(bf16)
    # wv groups: wv[g][d_local, k2*512 + vf] = Wv[vf, 128*(4g+k2) + d_local]
    wv_h = np.ascontiguousarray(
        qkv_w[2560:3072].T.reshape(4, 4, 128, 512).transpose(0, 2, 1, 3).reshape(4, 128, D)
    ).astype(bf16)
    wo_h = np.ascontiguousarray(out_w.T).reshape(16, 128, D).astype(bf16)
    qbias_h = np.ascontiguousarray(qkv_b[:2048].reshape(16, 128).T)
    kbias_h = np.ascontiguousarray(qkv_b[2048:2560].reshape(4, 128).T)
    vbias_h = qkv_b[2560:3072].reshape(1, 512).astype(bf16)
    outb_h = out_b.reshape(1, D).astype(bf16)
    es = np.exp(sinks).reshape(NQF, 2)          # [head pair, h]
    esinkc_h = np.ascontiguousarray(np.tile(es.T, (4, 1)))  # [8, 16] rows (2qb+h)
    selqb_h = np.zeros((8, NQB, 128), np.float32)
    for qb in range(NQB):
        selqb_h[2 * qb, qb, 0:64] = 1.0
        selqb_h[2 * qb + 1, qb, 64:128] = 1.0
    selqb_h = selqb_h.astype(bf16)
    ones128_h = np.ones((128, 1), bf16)
    onescol_h = np.ones((1, 128), bf16)

    r = np.arange(128)[:, None]
    c = np.arange(128)[None, :]
    mprev = (r > c).astype(np.float32)
    mdiag = (r <= c).astype(np.float32)
    maskt_h = np.concatenate([mprev, mprev, mdiag, mdiag], axis=1).astype(bf16)
    mask0_h0 = np.concatenate(
        [np.zeros((128, 256), np.float32), mdiag, mdiag], axis=1
    ).astype(bf16)

    def sin_tabs(s):
        """Half-shifted signed sin tables for the swap-free rotate-half.
        TA rows [32:64],[96:128] = -s.T (multiplies x2 into out rows 0:32);
        TB rows [0:32],[64:96] = +s.T (multiplies x1 into out rows 32:64)."""
        n = s.shape[0]
        z = np.zeros((32, n), np.float32)
        ta = np.concatenate([z, -s.T, z, -s.T], 0)
        tb = np.concatenate([s.T, z, s.T, z], 0)
        return ta.astype(bf16), tb.astype(bf16)

    in_maps = []
    for core in range(8):
        b, half = core // 2, core % 2
        t0 = half * TQ
        x_pad = np.zeros((TOK, D), np.float32)
        lo = t0 - HALO
        x_pad[max(0, -lo):] = x[b, max(lo, 0) : t0 + TQ]
        xt_h = np.ascontiguousarray(
            x_pad.T.reshape(NKT, 128, TOK).transpose(1, 0, 2)
        ).astype(bf16)
        cq, sq = _rope_tables(np.arange(t0, t0 + TQ))
        ck, sk = _rope_tables(np.clip(np.arange(t0 - HALO, t0 + TQ), 0, None))
        sqa, sqb = sin_tabs(sq)
        ska, skb = sin_tabs(sk)
        in_maps.append(
            {
                "xt": xt_h,
                "wq": wq_h,
                "wk": wk_h,
                "wv": wv_h,
                "wo": wo_h,
                "qbias": qbias_h,
                "kbias": kbias_h,
                "vbias": vbias_h,
                "outb": outb_h,
                "cosq": np.tile(np.concatenate([cq.T, cq.T], 0), (2, 1)).astype(bf16),
                "sinqa": sqa,
                "sinqb": sqb,
                "cosk": np.tile(np.concatenate([ck.T, ck.T], 0), (2, 1)).astype(bf16),
                "sinka": ska,
                "sinkb": skb,
                "maskt": maskt_h,
                "mask0": mask0_h0 if half == 0 else maskt_h,
                "esinkc": esinkc_h,
                "selqb": selqb_h,
                "ones128": ones128_h,
                "onescol": onescol_h,
            }
        )
    return in_maps


def kernel(x, qkv_w, qkv_b, out_w, out_b, sinks, _trace=False, _tmpdir=None):
    nc = _get_nc()
    in_maps = _host_inputs(x, qkv_w, qkv_b, out_w, out_b, sinks)
    kwargs = {}
    if _trace:
        kwargs = dict(trace=True, tmpdir=_tmpdir)
    res = run_bass_kernel_spmd(nc, in_maps, core_ids=list(range(8)), **kwargs)
    out = np.empty((B, T, D), np.float32)
    for core in range(8):
        b, half = core // 2, core % 2
        out[b, half * TQ : half * TQ + TQ] = res.results[core]["y"]
    if _trace:
        kernel._last_results = res
    return out
